# revision 1
# baseline (speedup 1.0000x reference)
"""AS_RNN (nn_AS_RNN_20134806683794) Trainium2 Bass kernel.

Recurrence (T=8192 steps, state R in R^4096, W in R^{4096x4096}):
    U_t = Lam * (W @ R_t + b) + (1-Lam) * y_t,   y_t = [x_t, 0_{3072}]
    R_{t+1} = tanh(U_t),   Lam = [0.5]*1024 ++ [1.0]*3072
    output = U_{T-1}[:1024]

Folded form used on device (host folds Lam into W rows / the additive term):
    U = Ws @ R + xp_t,   Ws = W with rows[:1024] * 0.5,
    xp_t = 0.5*(x_t + b[:1024]) ++ b[1024:]

8-core tensor-parallel K-sharding: core `me` holds the 512 W columns for its
R-chunk (8 MB fp32, resident in SBUF) and computes the partial contribution of
its chunk to all 8 output chunks per step: 32 fp32r matmuls
([128,1]^T x [128,512], K-accumulated in PSUM). Partials are folded to a
[128,32] vertical layout, AllReduce'd across the 8 cores (CCE does the sum),
and each core extracts its own 4-column block (partition_id-indexed dynamic
slice), adds xp, applies tanh on ACT, and writes its R chunk for the next
step.

Collectives cannot execute under hardware control flow, so the NEFF unrolls S
steps and the host invokes it T/S times; R state is carried in DRAM and all
large operands (W, XP) stay device-resident between invocations via a custom
shard_map executor.

Per-core layouts:
  - folded vertical [128,4] of a 512-chunk: element q = 4p+k at [p,k]
    (16-byte contiguous per partition -> single-descriptor fold DMA).
  - r_sb [128,4] fp32r holds R[512*me + 4p + k].
  - matmul k-tile k uses lhsT = r_sb[:,k:k+1] (K-subindices q=4p+k); the
    host-permuted W matches: rhs[p,n] of (k,j) = Ws[512*j+n, 512*me+4p+k].
"""

import sys

sys.path.insert(0, "/opt/trn_rl_repo")

import numpy as np

import concourse.bass as bass
import concourse.bacc as bacc
import concourse.tile as tile
import concourse.mybir as mybir

F32 = mybir.dt.float32
F32R = mybir.dt.float32r

NCORES = 8
CH = 512
KT = 4
VIS = 1024
T_FULL = 8192
S_STEPS = 256
LAST_EXEC_NS = -1  # steps unrolled per NEFF


def _host_prepare(X, W, b):
    """Per-core WT (permuted weights) and XP (additive term), host-side."""
    T = X.shape[0]
    Ws = W.astype(np.float32).copy()
    Ws[:VIS] *= 0.5
    W5 = Ws.reshape(8, 512, 8, 512)              # [l, n, me, q]
    WTs, XPs = [], []
    for me in range(8):
        A = W5[:, :, me, :]                      # [j, n, q]
        A = A.reshape(8, 512, 128, 4)            # [j, n, p, k]  (q = 4p+k)
        A = A.transpose(2, 3, 0, 1)              # [p, k, j, n]
        WTs.append(np.ascontiguousarray(A).reshape(128, KT * 8 * 512).astype(np.float32))
        if me < 2:
            xp = 0.5 * (X[:, CH * me:CH * (me + 1)] + b[CH * me:CH * (me + 1)])
        else:
            xp = np.broadcast_to(b[CH * me:CH * (me + 1)], (T, CH))
        XPv = xp.reshape(T, 128, 4).transpose(1, 0, 2)   # [p, t, k]
        XPs.append(np.ascontiguousarray(XPv).reshape(128, T * 4).astype(np.float32))
    return WTs, XPs


def _build(S, mm_dtype=F32R):
    """Bass program with S unrolled steps. Inputs WT/XP/RIN, outputs ROUT/UOUT."""
    nc = bacc.Bacc("TRN2", target_bir_lowering=False, debug=False,
                   num_devices=NCORES, detect_race_conditions=False)

    WT_d = nc.dram_tensor("WT", [128, KT * 8 * 512], mm_dtype, kind="ExternalInput").ap()
    XP_d = nc.dram_tensor("XP", [128, S * 4], F32, kind="ExternalInput").ap()
    RIN_d = nc.dram_tensor("RIN", [128, 4], mm_dtype, kind="ExternalInput").ap()
    RO_d = nc.dram_tensor("ROUT", [128, 4], mm_dtype, kind="ExternalOutput").ap()
    UO_d = nc.dram_tensor("UOUT", [128, 4], F32, kind="ExternalOutput").ap()

    wt_sb = nc.alloc_sbuf_tensor("wt_sb", [128, KT * 8 * 512], mm_dtype).ap()
    xp_sb = nc.alloc_sbuf_tensor("xp_sb", [128, S * 4], F32).ap()
    r_sb = nc.alloc_sbuf_tensor("r_sb", [128, 4], mm_dtype).ap()
    u_sb = nc.alloc_sbuf_tensor("u_sb", [128, 4], F32).ap()
    hp = [nc.alloc_sbuf_tensor(f"hp{p}", [1, 8 * 512], F32).ap() for p in range(2)]
    rb = [nc.alloc_sbuf_tensor(f"rb{p}", [128, 4], F32).ap() for p in range(2)]

    with tile.TileContext(nc) as tc:
        me = nc.partition_id()
        for c in range(8):
            sl = slice(c * 2048, (c + 1) * 2048)
            nc.scalar.dma_start(out=wt_sb[:, sl], in_=WT_d[:, sl])
        nc.scalar.dma_start(out=xp_sb, in_=XP_d)
        nc.sync.dma_start(out=r_sb, in_=RIN_d)

        with tc.tile_pool(name="dram", bufs=2, space="DRAM") as dram, \
             tc.tile_pool(name="ps", bufs=1, space="PSUM") as ps_pool:
            for s in range(S):
                par = s % 2
                cin = dram.tile([128, 32], F32, tag="cin")
                cout = dram.tile([128, 32], F32, tag="cout")
                for j in range(8):
                    ps = ps_pool.tile([1, 512], F32, tag=f"ps{j}")
                    for k in range(KT):
                        nc.tensor.matmul(
                            ps[0:1, :],
                            lhsT=r_sb[:, k:k + 1],
                            rhs=wt_sb[:, (k * 8 + j) * 512:(k * 8 + j + 1) * 512],
                            start=(k == 0),
                            stop=(k == KT - 1),
                        )
                    h = hp[par][0:1, j * 512:(j + 1) * 512]
                    if j % 2 == 0:
                        nc.vector.tensor_copy(h, ps[0:1, :])
                    else:
                        nc.scalar.activation(h, ps[0:1, :],
                                             mybir.ActivationFunctionType.Copy)
                    nc.sync.dma_start(
                        out=cin[:, 4 * j:4 * j + 4],
                        in_=h.rearrange("a (p k) -> a p k", p=128),
                    )
                nc.gpsimd.collective_compute(
                    "AllReduce", mybir.AluOpType.add,
                    replica_groups=[list(range(NCORES))],
                    ins=[cin[:].opt()], outs=[cout[:].opt()],
                )
                nc.sync.dma_start(out=rb[par], in_=cout[:, bass.ds(me * 4, 4)])
                nc.vector.tensor_add(u_sb, rb[par], xp_sb[:, 4 * s:4 * s + 4])
                nc.scalar.activation(r_sb, u_sb, mybir.ActivationFunctionType.Tanh)

        nc.sync.dma_start(out=RO_d, in_=r_sb)
        nc.sync.dma_start(out=UO_d, in_=u_sb)

    nc.compile()
    return nc


def _make_runner(nc):
    """jit'd shard_map executor over 8 cores mirroring bass2jax's
    run_bass_via_pjrt, but reusable with device-resident inputs."""
    import jax
    from jax.experimental.shard_map import shard_map
    from jax.sharding import Mesh, PartitionSpec
    from concourse.bass2jax import (
        _bass_exec_p, install_neuronx_cc_hook, partition_id_tensor,
    )

    install_neuronx_cc_hook()
    partition_name = nc.partition_id_tensor.name if nc.partition_id_tensor else None
    in_names, out_names, out_avals, zero_shapes = [], [], [], []
    for alloc in nc.m.functions[0].allocations:
        if not isinstance(alloc, mybir.MemoryLocationSet):
            continue
        name = alloc.memorylocations[0].name
        if alloc.kind == "ExternalInput":
            if name != partition_name:
                in_names.append(name)
        elif alloc.kind == "ExternalOutput":
            out_names.append(name)
            shape = tuple(alloc.tensor_shape)
            dtype = mybir.dt.np(alloc.dtype)
            out_avals.append(jax.core.ShapedArray(shape, dtype))
            zero_shapes.append((shape, dtype))
    n_params = len(in_names)
    all_in_names = list(in_names) + list(out_names)
    if partition_name is not None:
        all_in_names.append(partition_name)
    donate = tuple(range(n_params, n_params + len(out_names)))

    def _body(*args):
        operands = list(args)
        if partition_name is not None:
            operands.append(partition_id_tensor())
        outs = _bass_exec_p.bind(
            *operands,
            out_avals=tuple(out_avals),
            in_names=tuple(all_in_names),
            out_names=tuple(out_names),
            lowering_input_output_aliases=(),
            sim_require_finite=True,
            sim_require_nnan=True,
            nc=nc,
        )
        return tuple(outs)

    devices = jax.devices()[:NCORES]
    mesh = Mesh(np.asarray(devices), ("core",))
    in_specs = (PartitionSpec("core"),) * (n_params + len(out_names))
    out_specs = (PartitionSpec("core"),) * len(out_names)
    sharded = jax.jit(
        shard_map(_body, mesh=mesh, in_specs=in_specs, out_specs=out_specs,
                  check_rep=False),
        donate_argnums=donate, keep_unused=True,
    )
    return sharded, mesh, in_names, out_names, zero_shapes


def kernel(X, W, b):
    import jax
    from jax.sharding import NamedSharding, PartitionSpec

    T = X.shape[0]
    S = S_STEPS
    assert T % S == 0
    NB = T // S

    nc = _build(S)
    sharded, mesh, in_names, out_names, zero_shapes = _make_runner(nc)
    assert in_names == ["WT", "XP", "RIN"], in_names
    assert out_names == ["ROUT", "UOUT"], out_names

    WTs, XPs = _host_prepare(X, W, b)
    shard = NamedSharding(mesh, PartitionSpec("core"))
    WTg = jax.device_put(np.concatenate(WTs, axis=0), shard)
    XPcat = np.concatenate(XPs, axis=0)                        # [1024, T*4]
    XP_blocks = [
        jax.device_put(np.ascontiguousarray(XPcat[:, i * S * 4:(i + 1) * S * 4]), shard)
        for i in range(NB)
    ]
    R = jax.device_put(np.zeros((NCORES * 128, 4), np.float32), shard)

    import time as _time
    global LAST_EXEC_NS
    uout = None
    inv_times = []
    for i in range(NB):
        t0 = _time.time()
        zeros = [jax.device_put(np.zeros((NCORES * s[0], *s[1:]), d), shard)
                 for (s, d) in zero_shapes]
        R, uout = sharded(WTg, XP_blocks[i], R, *zeros)
        jax.block_until_ready(R)       # keep the device queue shallow
        inv_times.append(_time.time() - t0)
    ufull = np.asarray(uout)           # [1024, 4]; core c rows 128c:128c+128
    if len(inv_times) > 2:
        LAST_EXEC_NS = int(np.median(inv_times[2:]) * NB * 1e9)

    out = np.zeros(VIS, np.float32)
    for c in range(2):
        out[CH * c:CH * (c + 1)] = ufull[128 * c:128 * (c + 1)].reshape(CH)
    return out



# revision 3
# speedup vs baseline: 2.5465x; 2.5465x over previous
"""AS_RNN (nn_AS_RNN_20134806683794) Trainium2 Bass kernel — v1 row-parallel.

Recurrence (T=8192 steps, state R in R^4096, W in R^{4096x4096}):
    U_t = Lam * (W @ R_t + b) + (1-Lam) * y_t,   y_t = [x_t, 0_{3072}]
    R_{t+1} = tanh(U_t),   Lam = [0.5]*1024 ++ [1.0]*3072
    output = U_{T-1}[:1024]

Folded on device: U = Ws @ R + xp_t with Ws = W rows[:1024] * 0.5 and
xp_t = 0.5*(x_t + b[:1024]) ++ b[1024:].

8-core row-parallel (N-shard): core `me` owns output rows
[512*me, 512*me+512). Per step it contracts the FULL R (gathered in
lhsT layout r_sb[p, c] = R[32p+c]) against its host-permuted W block
(rhs[p, c*512+n] = Ws[512*me+n, 32p+c]) as 32 accumulating
[128,1]x[128,512] fp32r matmuls into one PSUM bank, adds xp via one
extra K=1 matmul (rank-1 update, rhs streamed from an SBUF xp buffer
prefetched from DRAM in 16-step blocks), applies tanh on ACT
(PSUM -> SBUF [1,512]), DMAs the 2KB chunk to DRAM, and AllGathers the
8 chunks; the gathered [128,32] is DMA'd back as next step's lhsT
(partition-contiguous, 128B per partition).

Collectives cannot run under HW control flow, so the NEFF unrolls
S_STEPS steps and the host invokes it T/S times. All invocations are
queued asynchronously (no per-call block_until_ready — the axon
round-trip is ~70ms) with outputs donated from pre-staged zero buffers;
R state chains device-side through ROUT->RIN.
"""

import sys

sys.path.insert(0, "/opt/trn_rl_repo")

import numpy as np

import concourse.bass as bass
import concourse.bacc as bacc
import concourse.tile as tile
import concourse.mybir as mybir

F32 = mybir.dt.float32
F32R = mybir.dt.float32r

NCORES = 8
CH = 512
VIS = 1024
T_FULL = 8192
S_STEPS = 512
PF = 16            # xp prefetch block (steps)
LAST_EXEC_NS = -1


def _host_prepare(X, W, b):
    """Per-core permuted weights WT and additive-term XP, host-side.

    WT[me][p, c*512+n] = Ws[512*me+n, 32*p+c]   (Ws = W, rows[:VIS] * 0.5)
    XP[me][0, t*512+n] = 0.5*(X[t,i]+b[i]) if i<VIS else b[i],  i = 512*me+n
    """
    T = X.shape[0]
    Ws = W.astype(np.float32).copy()
    Ws[:VIS] *= 0.5
    A = Ws.reshape(8, 512, 128, 32).transpose(0, 2, 3, 1)  # [me, p, c, n]
    WTs = [np.ascontiguousarray(A[me]).reshape(128, 32 * 512).astype(np.float32)
           for me in range(8)]
    XPs = []
    for me in range(8):
        lo, hi = CH * me, CH * (me + 1)
        if hi <= VIS:
            xp = 0.5 * (X[:, lo:hi] + b[lo:hi])
        else:
            xp = np.broadcast_to(b[lo:hi], (T, CH))
        XPs.append(np.ascontiguousarray(xp).reshape(1, T * CH).astype(np.float32))
    return WTs, XPs


def _build(S):
    """Bass program with S unrolled steps.

    Inputs WT [128, 32*512], XP [1, S*512], RIN [128, 32];
    outputs ROUT [128, 32], UOUT [1, 512].
    """
    nc = bacc.Bacc("TRN2", target_bir_lowering=False, debug=False,
                   num_devices=NCORES, detect_race_conditions=False)

    WT_d = nc.dram_tensor("WT", [128, 32 * 512], F32R, kind="ExternalInput").ap()
    XP_d = nc.dram_tensor("XP", [1, S * 512], F32R, kind="ExternalInput").ap()
    RIN_d = nc.dram_tensor("RIN", [128, 32], F32R, kind="ExternalInput").ap()
    RO_d = nc.dram_tensor("ROUT", [128, 32], F32R, kind="ExternalOutput").ap()
    UO_d = nc.dram_tensor("UOUT", [1, 512], F32, kind="ExternalOutput").ap()

    wt_sb = nc.alloc_sbuf_tensor("wt_sb", [128, 32 * 512], F32R).ap()
    one_sb = nc.alloc_sbuf_tensor("one_sb", [1, 1], F32R).ap()
    one_f32 = nc.alloc_sbuf_tensor("one_f32", [1, 1], F32).ap()

    with tile.TileContext(nc) as tc:
        for q in range(8):
            sl = slice(q * 2048, (q + 1) * 2048)
            nc.scalar.dma_start(out=wt_sb[:, sl], in_=WT_d[:, sl])
        nc.gpsimd.memset(one_f32, 1.0)
        nc.vector.tensor_copy(one_sb, one_f32)

        with tc.tile_pool(name="dram", bufs=2, space="DRAM") as dram, \
             tc.tile_pool(name="ps_pool", bufs=2, space="PSUM") as ps_pool, \
             tc.tile_pool(name="sbp", bufs=2) as sbp, \
             tc.tile_pool(name="xpp", bufs=2) as xpp:
            r_cur = None
            xp_blk = None
            for s in range(S):
                if s % PF == 0:
                    n_pf = min(PF, S - s)
                    xp_blk = xpp.tile([1, PF * 512], F32R, tag="xpb")
                    nc.sync.dma_start(
                        out=xp_blk[0:1, :n_pf * 512],
                        in_=XP_d[0:1, s * 512:(s + n_pf) * 512])
                if s == 0:
                    r_cur = sbp.tile([128, 32], F32R, tag="rsb")
                    nc.sync.dma_start(out=r_cur, in_=RIN_d)
                ps = ps_pool.tile([1, 512], F32, tag="ps")
                for c in range(32):
                    nc.tensor.matmul(
                        ps[0:1, :],
                        lhsT=r_cur[:, c:c + 1],
                        rhs=wt_sb[:, c * 512:(c + 1) * 512],
                        start=(c == 0), stop=False)
                nc.tensor.matmul(
                    ps[0:1, :],
                    lhsT=one_sb[0:1, 0:1],
                    rhs=xp_blk[0:1, (s % PF) * 512:(s % PF + 1) * 512],
                    start=False, stop=True)
                r_out = sbp.tile([1, 512], F32R, tag="rout")
                nc.scalar.activation(r_out, ps[0:1, :],
                                     mybir.ActivationFunctionType.Tanh)
                cin = dram.tile([1, 512], F32R, tag="cin")
                cout = dram.tile([128, 32], F32R, tag="cout")
                nc.sync.dma_start(out=cin, in_=r_out)
                nc.gpsimd.collective_compute(
                    "AllGather", mybir.AluOpType.bypass,
                    replica_groups=[list(range(NCORES))],
                    ins=[cin[:].opt()], outs=[cout[:].opt()])
                if s < S - 1:
                    r_cur = sbp.tile([128, 32], F32R, tag="rsb")
                    nc.sync.dma_start(out=r_cur, in_=cout[:])
                else:
                    u_sb = sbp.tile([1, 512], F32, tag="usb")
                    nc.vector.tensor_copy(u_sb, ps[0:1, :])
                    nc.sync.dma_start(out=RO_d, in_=cout[:])
                    nc.sync.dma_start(out=UO_d, in_=u_sb)

    nc.compile()
    return nc


def _make_runner(nc):
    """jit'd shard_map executor over 8 cores (mirrors bass2jax's
    run_bass_via_pjrt but reusable with device-resident inputs)."""
    import jax
    from jax.experimental.shard_map import shard_map
    from jax.sharding import Mesh, PartitionSpec
    from concourse.bass2jax import (
        _bass_exec_p, install_neuronx_cc_hook, partition_id_tensor,
    )

    install_neuronx_cc_hook()
    partition_name = nc.partition_id_tensor.name if nc.partition_id_tensor else None
    in_names, out_names, out_avals, zero_shapes = [], [], [], []
    for alloc in nc.m.functions[0].allocations:
        if not isinstance(alloc, mybir.MemoryLocationSet):
            continue
        name = alloc.memorylocations[0].name
        if alloc.kind == "ExternalInput":
            if name != partition_name:
                in_names.append(name)
        elif alloc.kind == "ExternalOutput":
            out_names.append(name)
            shape = tuple(alloc.tensor_shape)
            dtype = mybir.dt.np(alloc.dtype)
            out_avals.append(jax.core.ShapedArray(shape, dtype))
            zero_shapes.append((shape, dtype))
    n_params = len(in_names)
    all_in_names = list(in_names) + list(out_names)
    if partition_name is not None:
        all_in_names.append(partition_name)
    donate = tuple(range(n_params, n_params + len(out_names)))

    def _body(*args):
        operands = list(args)
        if partition_name is not None:
            operands.append(partition_id_tensor())
        outs = _bass_exec_p.bind(
            *operands,
            out_avals=tuple(out_avals),
            in_names=tuple(all_in_names),
            out_names=tuple(out_names),
            lowering_input_output_aliases=(),
            sim_require_finite=True,
            sim_require_nnan=True,
            nc=nc,
        )
        return tuple(outs)

    devices = jax.devices()[:NCORES]
    mesh = Mesh(np.asarray(devices), ("core",))
    in_specs = (PartitionSpec("core"),) * (n_params + len(out_names))
    out_specs = (PartitionSpec("core"),) * len(out_names)
    sharded = jax.jit(
        shard_map(_body, mesh=mesh, in_specs=in_specs, out_specs=out_specs,
                  check_rep=False),
        donate_argnums=donate, keep_unused=True,
    )
    return sharded, mesh, in_names, out_names, zero_shapes


def kernel(X, W, b):
    import jax
    import time as _time
    from jax.sharding import NamedSharding, PartitionSpec

    T = X.shape[0]
    S = min(S_STEPS, T)
    assert T % S == 0
    NB = T // S

    nc = _build(S)
    sharded, mesh, in_names, out_names, zero_shapes = _make_runner(nc)
    assert in_names == ["WT", "XP", "RIN"], in_names
    assert out_names == ["ROUT", "UOUT"], out_names

    WTs, XPs = _host_prepare(X, W, b)
    shard = NamedSharding(mesh, PartitionSpec("core"))
    WTg = jax.device_put(np.concatenate(WTs, axis=0), shard)
    XPcat = np.concatenate(XPs, axis=0)                     # [8, T*512]
    XP_blocks = [
        jax.device_put(
            np.ascontiguousarray(XPcat[:, i * S * 512:(i + 1) * S * 512]), shard)
        for i in range(NB)
    ]
    R = jax.device_put(np.zeros((NCORES * 128, 32), np.float32), shard)
    zeros_all = [
        [jax.device_put(np.zeros((NCORES * s[0], *s[1:]), d), shard)
         for (s, d) in zero_shapes]
        for _ in range(NB)
    ]
    jax.block_until_ready(WTg)
    jax.block_until_ready(zeros_all[-1])

    global LAST_EXEC_NS
    uout = None
    t0 = _time.time()
    for i in range(NB):
        R, uout = sharded(WTg, XP_blocks[i], R, *zeros_all[i])
    jax.block_until_ready(uout)
    LAST_EXEC_NS = int((_time.time() - t0) * 1e9)

    ufull = np.asarray(uout)            # [8, 512]; core c -> U[512c:512c+512]
    out = np.zeros(VIS, np.float32)
    for c in range(2):
        out[CH * c:CH * (c + 1)] = ufull[c]
    return out


# revision 5
# speedup vs baseline: 12.5902x; 4.9441x over previous
"""AS_RNN (nn_AS_RNN_20134806683794) Trainium2 Bass kernel — v1 row-parallel.

Recurrence (T=8192 steps, state R in R^4096, W in R^{4096x4096}):
    U_t = Lam * (W @ R_t + b) + (1-Lam) * y_t,   y_t = [x_t, 0_{3072}]
    R_{t+1} = tanh(U_t),   Lam = [0.5]*1024 ++ [1.0]*3072
    output = U_{T-1}[:1024]

Folded on device: U = Ws @ R + xp_t with Ws = W rows[:1024] * 0.5 and
xp_t = 0.5*(x_t + b[:1024]) ++ b[1024:].

8-core row-parallel (N-shard): core `me` owns output rows
[512*me, 512*me+512). Per step it contracts the FULL R (gathered in
lhsT layout r_sb[p, c] = R[32p+c]) against its host-permuted W block
(rhs[p, c*512+n] = Ws[512*me+n, 32p+c]) as 32 accumulating
[128,1]x[128,512] fp32r matmuls into one PSUM bank, adds xp via one
extra K=1 matmul (rank-1 update, rhs streamed from an SBUF xp buffer
prefetched from DRAM in 16-step blocks), applies tanh on ACT
(PSUM -> SBUF [1,512]), DMAs the 2KB chunk to DRAM, and AllGathers the
8 chunks; the gathered [128,32] is DMA'd back as next step's lhsT
(partition-contiguous, 128B per partition).

Collectives cannot run under HW control flow, so the NEFF unrolls
S_STEPS steps and the host invokes it T/S times. All invocations are
queued asynchronously (no per-call block_until_ready — the axon
round-trip is ~70ms) with outputs donated from pre-staged zero buffers;
R state chains device-side through ROUT->RIN.
"""

import sys

sys.path.insert(0, "/opt/trn_rl_repo")

import numpy as np

import concourse.bass as bass
import concourse.bacc as bacc
import concourse.tile as tile
import concourse.mybir as mybir

F32 = mybir.dt.float32
F32R = mybir.dt.float32r

NCORES = 8
CH = 512
VIS = 1024
T_FULL = 8192
S_STEPS = 2048
PF = 16            # xp prefetch block (steps)
LAST_EXEC_NS = -1


def _host_prepare(X, W, b):
    """Per-core permuted weights WT and additive-term XP, host-side.

    WT[me][p, c*512+n] = Ws[512*me+n, 32*p+c]   (Ws = W, rows[:VIS] * 0.5)
    XP[me][0, t*512+n] = 0.5*(X[t,i]+b[i]) if i<VIS else b[i],  i = 512*me+n
    """
    T = X.shape[0]
    Ws = W.astype(np.float32).copy()
    Ws[:VIS] *= 0.5
    A = Ws.reshape(8, 512, 128, 32).transpose(0, 2, 3, 1)  # [me, p, c, n]
    WTs = [np.ascontiguousarray(A[me]).reshape(128, 32 * 512).astype(np.float32)
           for me in range(8)]
    XPs = []
    for me in range(8):
        lo, hi = CH * me, CH * (me + 1)
        if hi <= VIS:
            xp = 0.5 * (X[:, lo:hi] + b[lo:hi])
        else:
            xp = np.broadcast_to(b[lo:hi], (T, CH))
        XPs.append(np.ascontiguousarray(xp).reshape(1, T * CH).astype(np.float32))
    return WTs, XPs


def _build(S):
    """Bass program with S unrolled steps.

    Inputs WT [128, 32*512], XP [1, S*512], RIN [128, 32];
    outputs ROUT [128, 32], UOUT [1, 512].
    """
    nc = bacc.Bacc("TRN2", target_bir_lowering=False, debug=False,
                   num_devices=NCORES, detect_race_conditions=False)

    WT_d = nc.dram_tensor("WT", [128, 32 * 512], F32R, kind="ExternalInput").ap()
    XP_d = nc.dram_tensor("XP", [1, S * 512], F32R, kind="ExternalInput").ap()
    RIN_d = nc.dram_tensor("RIN", [128, 32], F32R, kind="ExternalInput").ap()
    RO_d = nc.dram_tensor("ROUT", [128, 32], F32R, kind="ExternalOutput").ap()
    UO_d = nc.dram_tensor("UOUT", [1, 512], F32, kind="ExternalOutput").ap()

    wt_sb = nc.alloc_sbuf_tensor("wt_sb", [128, 32 * 512], F32R).ap()
    one_sb = nc.alloc_sbuf_tensor("one_sb", [1, 1], F32R).ap()
    one_f32 = nc.alloc_sbuf_tensor("one_f32", [1, 1], F32).ap()

    with tile.TileContext(nc) as tc:
        for q in range(8):
            sl = slice(q * 2048, (q + 1) * 2048)
            nc.scalar.dma_start(out=wt_sb[:, sl], in_=WT_d[:, sl])
        nc.gpsimd.memset(one_f32, 1.0)
        nc.vector.tensor_copy(one_sb, one_f32)

        with tc.tile_pool(name="dram", bufs=2, space="DRAM") as dram, \
             tc.tile_pool(name="ps_pool", bufs=2, space="PSUM") as ps_pool, \
             tc.tile_pool(name="sbp", bufs=2) as sbp, \
             tc.tile_pool(name="xpp", bufs=2) as xpp:
            r_cur = None
            xp_blk = None
            for s in range(S):
                if s % PF == 0:
                    n_pf = min(PF, S - s)
                    xp_blk = xpp.tile([1, PF * 512], F32R, tag="xpb")
                    nc.sync.dma_start(
                        out=xp_blk[0:1, :n_pf * 512],
                        in_=XP_d[0:1, s * 512:(s + n_pf) * 512])
                if s == 0:
                    r_cur = sbp.tile([128, 32], F32R, tag="rsb")
                    nc.sync.dma_start(out=r_cur, in_=RIN_d)
                ps = ps_pool.tile([1, 512], F32, tag="ps")
                for c in range(32):
                    nc.tensor.matmul(
                        ps[0:1, :],
                        lhsT=r_cur[:, c:c + 1],
                        rhs=wt_sb[:, c * 512:(c + 1) * 512],
                        start=(c == 0), stop=False)
                nc.tensor.matmul(
                    ps[0:1, :],
                    lhsT=one_sb[0:1, 0:1],
                    rhs=xp_blk[0:1, (s % PF) * 512:(s % PF + 1) * 512],
                    start=False, stop=True)
                r_out = sbp.tile([1, 512], F32R, tag="rout")
                nc.scalar.activation(r_out, ps[0:1, :],
                                     mybir.ActivationFunctionType.Tanh)
                cin = dram.tile([1, 512], F32R, tag="cin")
                cout = dram.tile([128, 32], F32R, tag="cout")
                nc.sync.dma_start(out=cin, in_=r_out)
                nc.gpsimd.collective_compute(
                    "AllGather", mybir.AluOpType.bypass,
                    replica_groups=[list(range(NCORES))],
                    ins=[cin[:].opt()], outs=[cout[:].opt()])
                if s < S - 1:
                    r_cur = sbp.tile([128, 32], F32R, tag="rsb")
                    nc.sync.dma_start(out=r_cur, in_=cout[:])
                else:
                    u_sb = sbp.tile([1, 512], F32, tag="usb")
                    nc.vector.tensor_copy(u_sb, ps[0:1, :])
                    nc.sync.dma_start(out=RO_d, in_=cout[:])
                    nc.sync.dma_start(out=UO_d, in_=u_sb)

    nc.compile()
    return nc


def _make_runner(nc):
    """jit'd shard_map executor over 8 cores (mirrors bass2jax's
    run_bass_via_pjrt but reusable with device-resident inputs)."""
    import jax
    from jax.experimental.shard_map import shard_map
    from jax.sharding import Mesh, PartitionSpec
    from concourse.bass2jax import (
        _bass_exec_p, install_neuronx_cc_hook, partition_id_tensor,
    )

    install_neuronx_cc_hook()
    partition_name = nc.partition_id_tensor.name if nc.partition_id_tensor else None
    in_names, out_names, out_avals, zero_shapes = [], [], [], []
    for alloc in nc.m.functions[0].allocations:
        if not isinstance(alloc, mybir.MemoryLocationSet):
            continue
        name = alloc.memorylocations[0].name
        if alloc.kind == "ExternalInput":
            if name != partition_name:
                in_names.append(name)
        elif alloc.kind == "ExternalOutput":
            out_names.append(name)
            shape = tuple(alloc.tensor_shape)
            dtype = mybir.dt.np(alloc.dtype)
            out_avals.append(jax.core.ShapedArray(shape, dtype))
            zero_shapes.append((shape, dtype))
    n_params = len(in_names)
    all_in_names = list(in_names) + list(out_names)
    if partition_name is not None:
        all_in_names.append(partition_name)
    donate = tuple(range(n_params, n_params + len(out_names)))

    def _body(*args):
        operands = list(args)
        if partition_name is not None:
            operands.append(partition_id_tensor())
        outs = _bass_exec_p.bind(
            *operands,
            out_avals=tuple(out_avals),
            in_names=tuple(all_in_names),
            out_names=tuple(out_names),
            lowering_input_output_aliases=(),
            sim_require_finite=True,
            sim_require_nnan=True,
            nc=nc,
        )
        return tuple(outs)

    devices = jax.devices()[:NCORES]
    mesh = Mesh(np.asarray(devices), ("core",))
    in_specs = (PartitionSpec("core"),) * (n_params + len(out_names))
    out_specs = (PartitionSpec("core"),) * len(out_names)
    sharded = jax.jit(
        shard_map(_body, mesh=mesh, in_specs=in_specs, out_specs=out_specs,
                  check_rep=False),
        donate_argnums=donate, keep_unused=True,
    )
    return sharded, mesh, in_names, out_names, zero_shapes


def kernel(X, W, b):
    import jax
    import time as _time
    from jax.sharding import NamedSharding, PartitionSpec

    T = X.shape[0]
    S = min(S_STEPS, T)
    assert T % S == 0
    NB = T // S

    nc = _build(S)
    sharded, mesh, in_names, out_names, zero_shapes = _make_runner(nc)
    assert in_names == ["WT", "XP", "RIN"], in_names
    assert out_names == ["ROUT", "UOUT"], out_names

    WTs, XPs = _host_prepare(X, W, b)
    shard = NamedSharding(mesh, PartitionSpec("core"))
    WTg = jax.device_put(np.concatenate(WTs, axis=0), shard)
    XPcat = np.concatenate(XPs, axis=0)                     # [8, T*512]
    XP_blocks = [
        jax.device_put(
            np.ascontiguousarray(XPcat[:, i * S * 512:(i + 1) * S * 512]), shard)
        for i in range(NB)
    ]
    R = jax.device_put(np.zeros((NCORES * 128, 32), np.float32), shard)
    zeros_all = [
        [jax.device_put(np.zeros((NCORES * s[0], *s[1:]), d), shard)
         for (s, d) in zero_shapes]
        for _ in range(NB + 1)
    ]
    jax.block_until_ready(WTg)
    jax.block_until_ready(zeros_all[-1])

    # Untimed warmup: loads the NEFF onto the cores and fills runtime
    # caches; outputs are discarded (R state for the real run starts
    # fresh from zeros below).
    Rw = jax.device_put(np.zeros((NCORES * 128, 32), np.float32), shard)
    Rw, uw = sharded(WTg, XP_blocks[0], Rw, *zeros_all[NB])
    jax.block_until_ready(uw)

    global LAST_EXEC_NS
    uout = None
    t0 = _time.time()
    for i in range(NB):
        R, uout = sharded(WTg, XP_blocks[i], R, *zeros_all[i])
    jax.block_until_ready(uout)
    LAST_EXEC_NS = int((_time.time() - t0) * 1e9)

    ufull = np.asarray(uout)            # [8, 512]; core c -> U[512c:512c+512]
    out = np.zeros(VIS, np.float32)
    for c in range(2):
        out[CH * c:CH * (c + 1)] = ufull[c]
    return out


# revision 6
# speedup vs baseline: 12.6715x; 1.0065x over previous
"""AS_RNN (nn_AS_RNN_20134806683794) Trainium2 Bass kernel — v1 row-parallel.

Recurrence (T=8192 steps, state R in R^4096, W in R^{4096x4096}):
    U_t = Lam * (W @ R_t + b) + (1-Lam) * y_t,   y_t = [x_t, 0_{3072}]
    R_{t+1} = tanh(U_t),   Lam = [0.5]*1024 ++ [1.0]*3072
    output = U_{T-1}[:1024]

Folded on device: U = Ws @ R + xp_t with Ws = W rows[:1024] * 0.5 and
xp_t = 0.5*(x_t + b[:1024]) ++ b[1024:].

8-core row-parallel (N-shard): core `me` owns output rows
[512*me, 512*me+512). Per step it contracts the FULL R (gathered in
lhsT layout r_sb[p, c] = R[32p+c]) against its host-permuted W block
(rhs[p, c*512+n] = Ws[512*me+n, 32p+c]) as 32 accumulating
[128,1]x[128,512] fp32r matmuls into one PSUM bank, adds xp via one
extra K=1 matmul (rank-1 update, rhs streamed from an SBUF xp buffer
prefetched from DRAM in 16-step blocks), applies tanh on ACT
(PSUM -> SBUF [1,512]), DMAs the 2KB chunk to DRAM, and AllGathers the
8 chunks; the gathered [128,32] is DMA'd back as next step's lhsT
(partition-contiguous, 128B per partition).

Collectives cannot run under HW control flow, so the NEFF unrolls
S_STEPS steps and the host invokes it T/S times. All invocations are
queued asynchronously (no per-call block_until_ready — the axon
round-trip is ~70ms) with outputs donated from pre-staged zero buffers;
R state chains device-side through ROUT->RIN.
"""

import sys

sys.path.insert(0, "/opt/trn_rl_repo")

import numpy as np

import concourse.bass as bass
import concourse.bacc as bacc
import concourse.tile as tile
import concourse.mybir as mybir

F32 = mybir.dt.float32
F32R = mybir.dt.float32r

NCORES = 8
CH = 512
VIS = 1024
T_FULL = 8192
S_STEPS = 512
PF = 16            # xp prefetch block (steps)
LAST_EXEC_NS = -1


def _host_prepare(X, W, b):
    """Per-core permuted weights WT and additive-term XP, host-side.

    WT[me][p, c*512+n] = Ws[512*me+n, 32*p+c]   (Ws = W, rows[:VIS] * 0.5)
    XP[me][0, t*512+n] = 0.5*(X[t,i]+b[i]) if i<VIS else b[i],  i = 512*me+n
    """
    T = X.shape[0]
    Ws = W.astype(np.float32).copy()
    Ws[:VIS] *= 0.5
    A = Ws.reshape(8, 512, 128, 32).transpose(0, 2, 3, 1)  # [me, p, c, n]
    WTs = [np.ascontiguousarray(A[me]).reshape(128, 32 * 512).astype(np.float32)
           for me in range(8)]
    XPs = []
    for me in range(8):
        lo, hi = CH * me, CH * (me + 1)
        if hi <= VIS:
            xp = 0.5 * (X[:, lo:hi] + b[lo:hi])
        else:
            xp = np.broadcast_to(b[lo:hi], (T, CH))
        XPs.append(np.ascontiguousarray(xp).reshape(1, T * CH).astype(np.float32))
    return WTs, XPs


def _build(S):
    """Bass program with S unrolled steps.

    Inputs WT [128, 32*512], XP [1, S*512], RIN [128, 32];
    outputs ROUT [128, 32], UOUT [1, 512].
    """
    nc = bacc.Bacc("TRN2", target_bir_lowering=False, debug=False,
                   num_devices=NCORES, detect_race_conditions=False)

    WT_d = nc.dram_tensor("WT", [128, 32 * 512], F32R, kind="ExternalInput").ap()
    XP_d = nc.dram_tensor("XP", [1, S * 512], F32R, kind="ExternalInput").ap()
    RIN_d = nc.dram_tensor("RIN", [128, 32], F32R, kind="ExternalInput").ap()
    RO_d = nc.dram_tensor("ROUT", [128, 32], F32R, kind="ExternalOutput").ap()
    UO_d = nc.dram_tensor("UOUT", [1, 512], F32, kind="ExternalOutput").ap()

    wt_sb = nc.alloc_sbuf_tensor("wt_sb", [128, 32 * 512], F32R).ap()
    one_sb = nc.alloc_sbuf_tensor("one_sb", [1, 1], F32R).ap()
    one_f32 = nc.alloc_sbuf_tensor("one_f32", [1, 1], F32).ap()

    with tile.TileContext(nc) as tc:
        for q in range(8):
            sl = slice(q * 2048, (q + 1) * 2048)
            nc.scalar.dma_start(out=wt_sb[:, sl], in_=WT_d[:, sl])
        nc.gpsimd.memset(one_f32, 1.0)
        nc.vector.tensor_copy(one_sb, one_f32)

        with tc.tile_pool(name="dram", bufs=2, space="DRAM") as dram, \
             tc.tile_pool(name="ps_pool", bufs=2, space="PSUM") as ps_pool, \
             tc.tile_pool(name="sbp", bufs=2) as sbp, \
             tc.tile_pool(name="xpp", bufs=2) as xpp:
            r_cur = None
            xp_blk = None
            for s in range(S):
                if s % PF == 0:
                    n_pf = min(PF, S - s)
                    xp_blk = xpp.tile([1, PF * 512], F32R, tag="xpb")
                    nc.sync.dma_start(
                        out=xp_blk[0:1, :n_pf * 512],
                        in_=XP_d[0:1, s * 512:(s + n_pf) * 512])
                if s == 0:
                    r_cur = sbp.tile([128, 32], F32R, tag="rsb")
                    nc.sync.dma_start(out=r_cur, in_=RIN_d)
                ps = ps_pool.tile([1, 512], F32, tag="ps")
                for c in range(32):
                    nc.tensor.matmul(
                        ps[0:1, :],
                        lhsT=r_cur[:, c:c + 1],
                        rhs=wt_sb[:, c * 512:(c + 1) * 512],
                        start=(c == 0), stop=False)
                nc.tensor.matmul(
                    ps[0:1, :],
                    lhsT=one_sb[0:1, 0:1],
                    rhs=xp_blk[0:1, (s % PF) * 512:(s % PF + 1) * 512],
                    start=False, stop=True)
                r_out = sbp.tile([1, 512], F32R, tag="rout")
                nc.scalar.activation(r_out, ps[0:1, :],
                                     mybir.ActivationFunctionType.Tanh)
                cin = dram.tile([1, 512], F32R, tag="cin")
                cout = dram.tile([128, 32], F32R, tag="cout")
                nc.sync.dma_start(out=cin, in_=r_out)
                nc.gpsimd.collective_compute(
                    "AllGather", mybir.AluOpType.bypass,
                    replica_groups=[list(range(NCORES))],
                    ins=[cin[:].opt()], outs=[cout[:].opt()])
                if s < S - 1:
                    r_cur = sbp.tile([128, 32], F32R, tag="rsb")
                    nc.sync.dma_start(out=r_cur, in_=cout[:])
                else:
                    u_sb = sbp.tile([1, 512], F32, tag="usb")
                    nc.vector.tensor_copy(u_sb, ps[0:1, :])
                    nc.sync.dma_start(out=RO_d, in_=cout[:])
                    nc.sync.dma_start(out=UO_d, in_=u_sb)

    nc.compile()
    return nc


def _make_runner(nc):
    """jit'd shard_map executor over 8 cores (mirrors bass2jax's
    run_bass_via_pjrt but reusable with device-resident inputs)."""
    import jax
    from jax.experimental.shard_map import shard_map
    from jax.sharding import Mesh, PartitionSpec
    from concourse.bass2jax import (
        _bass_exec_p, install_neuronx_cc_hook, partition_id_tensor,
    )

    install_neuronx_cc_hook()
    partition_name = nc.partition_id_tensor.name if nc.partition_id_tensor else None
    in_names, out_names, out_avals, zero_shapes = [], [], [], []
    for alloc in nc.m.functions[0].allocations:
        if not isinstance(alloc, mybir.MemoryLocationSet):
            continue
        name = alloc.memorylocations[0].name
        if alloc.kind == "ExternalInput":
            if name != partition_name:
                in_names.append(name)
        elif alloc.kind == "ExternalOutput":
            out_names.append(name)
            shape = tuple(alloc.tensor_shape)
            dtype = mybir.dt.np(alloc.dtype)
            out_avals.append(jax.core.ShapedArray(shape, dtype))
            zero_shapes.append((shape, dtype))
    n_params = len(in_names)
    all_in_names = list(in_names) + list(out_names)
    if partition_name is not None:
        all_in_names.append(partition_name)
    donate = tuple(range(n_params, n_params + len(out_names)))

    def _body(*args):
        operands = list(args)
        if partition_name is not None:
            operands.append(partition_id_tensor())
        outs = _bass_exec_p.bind(
            *operands,
            out_avals=tuple(out_avals),
            in_names=tuple(all_in_names),
            out_names=tuple(out_names),
            lowering_input_output_aliases=(),
            sim_require_finite=True,
            sim_require_nnan=True,
            nc=nc,
        )
        return tuple(outs)

    devices = jax.devices()[:NCORES]
    mesh = Mesh(np.asarray(devices), ("core",))
    in_specs = (PartitionSpec("core"),) * (n_params + len(out_names))
    out_specs = (PartitionSpec("core"),) * len(out_names)
    sharded = jax.jit(
        shard_map(_body, mesh=mesh, in_specs=in_specs, out_specs=out_specs,
                  check_rep=False),
        donate_argnums=donate, keep_unused=True,
    )
    return sharded, mesh, in_names, out_names, zero_shapes


def kernel(X, W, b):
    import jax
    import time as _time
    from jax.sharding import NamedSharding, PartitionSpec

    T = X.shape[0]
    S = min(S_STEPS, T)
    assert T % S == 0
    NB = T // S

    nc = _build(S)
    sharded, mesh, in_names, out_names, zero_shapes = _make_runner(nc)
    assert in_names == ["WT", "XP", "RIN"], in_names
    assert out_names == ["ROUT", "UOUT"], out_names

    WTs, XPs = _host_prepare(X, W, b)
    shard = NamedSharding(mesh, PartitionSpec("core"))
    WTg = jax.device_put(np.concatenate(WTs, axis=0), shard)
    XPcat = np.concatenate(XPs, axis=0)                     # [8, T*512]
    XP_blocks = [
        jax.device_put(
            np.ascontiguousarray(XPcat[:, i * S * 512:(i + 1) * S * 512]), shard)
        for i in range(NB)
    ]
    R = jax.device_put(np.zeros((NCORES * 128, 32), np.float32), shard)
    zeros_all = [
        [jax.device_put(np.zeros((NCORES * s[0], *s[1:]), d), shard)
         for (s, d) in zero_shapes]
        for _ in range(NB + 1)
    ]
    jax.block_until_ready(WTg)
    jax.block_until_ready(zeros_all[-1])

    # Untimed warmup: loads the NEFF onto the cores and fills runtime
    # caches; outputs are discarded (R state for the real run starts
    # fresh from zeros below).
    Rw = jax.device_put(np.zeros((NCORES * 128, 32), np.float32), shard)
    Rw, uw = sharded(WTg, XP_blocks[0], Rw, *zeros_all[NB])
    jax.block_until_ready(uw)

    global LAST_EXEC_NS
    uout = None
    t0 = _time.time()
    for i in range(NB):
        R, uout = sharded(WTg, XP_blocks[i], R, *zeros_all[i])
    jax.block_until_ready(uout)
    LAST_EXEC_NS = int((_time.time() - t0) * 1e9)

    ufull = np.asarray(uout)            # [8, 512]; core c -> U[512c:512c+512]
    out = np.zeros(VIS, np.float32)
    for c in range(2):
        out[CH * c:CH * (c + 1)] = ufull[c]
    return out
